# revision 1
# baseline (speedup 1.0000x reference)
"""Single-head attention (InterModalAttention) Bass kernel for 8 TRN2 cores.

Sharding: batch (4) x query-half (2) -> 8 cores. Each core computes K/V for
its batch element (full 2048-seq) and attention for its 1024 queries.

Layout strategy (all matmuls contract over the partition dim):
  - Host pre-transposes x and weights -> xT [d,s], WT [d,e] so no on-chip
    transpose of inputs is needed.
  - qT/kT computed as [e, s] tiles (lhsT=WT tile, rhs=xT tile); bias added
    per-partition during PSUM->SBUF copyback.
  - v computed natural [s, e] (lhsT=xT tile, rhs=WvT tile); bias bv folded
    into the final epilogue (softmax rows sum to 1).
  - scores[i,j] psum accumulated over 8 e-tiles; exp on ACT engine with
    scale=1/32 and accum_out giving row-sums for free.
  - attn tiles PE-transposed (128x128) -> lhsT for out = attnT.T @ v,
    accumulated over 16 j-tiles in PSUM.
  - epilogue: out = psum * (1/rowsum) + bv.
All matmul operands use float32r (full-rate fp32 on the PE at N>=512).
"""
import sys
import numpy as np

for p in ("/opt/trn_rl_repo",):
    if p not in sys.path:
        sys.path.insert(0, p)

B, S, D = 4, 2048, 1024
NQ = 1024          # queries per core
NCORES = 8
P = 128
INV_SQRT_D = 1.0 / 32.0

_CACHE = {}


def build_nc():
    from contextlib import ExitStack
    import concourse.mybir as mybir
    import concourse.tile as tile
    from concourse import bacc
    from concourse.masks import make_identity

    F32 = mybir.dt.float32
    FR = mybir.dt.float32r
    AF = mybir.ActivationFunctionType

    nc = bacc.Bacc("TRN2", debug=False)

    xkvT = nc.dram_tensor("xkvT", (D, S), FR, kind="ExternalInput")
    xqT = nc.dram_tensor("xqT", (D, NQ), FR, kind="ExternalInput")
    wqT = nc.dram_tensor("wqT", (D, D), FR, kind="ExternalInput")
    wkT = nc.dram_tensor("wkT", (D, D), FR, kind="ExternalInput")
    wvT = nc.dram_tensor("wvT", (D, D), FR, kind="ExternalInput")
    bq = nc.dram_tensor("bq", (D,), F32, kind="ExternalInput")
    bk = nc.dram_tensor("bk", (D,), F32, kind="ExternalInput")
    bv = nc.dram_tensor("bv", (D,), F32, kind="ExternalInput")
    out = nc.dram_tensor("out", (NQ, D), F32, kind="ExternalOutput")

    ET = D // P            # 8 e-tiles
    DT = D // P            # 8 d-tiles
    SC = S // 512          # 4 s-chunks
    SB = S // P            # 16 s-blocks (j-tiles)
    IG = NQ // 512         # 2 i-groups
    EC = D // 512          # 2 e-chunks

    with tile.TileContext(nc) as tc, ExitStack() as ctx:
        consts = ctx.enter_context(tc.tile_pool(name="consts", bufs=1))
        ps512 = ctx.enter_context(tc.tile_pool(name="ps512", bufs=2, space="PSUM"))
        outps = ctx.enter_context(tc.tile_pool(name="outps", bufs=2, space="PSUM"))
        tpps = ctx.enter_context(tc.tile_pool(name="tpps", bufs=2, space="PSUM"))
        dram = ctx.enter_context(tc.tile_pool(name="dram", bufs=1, space="DRAM"))

        _eng = [nc.sync, nc.gpsimd, nc.scalar]
        _dmac = [0]
        def dma(out_ap, in_ap):
            e = _eng[_dmac[0] % len(_eng)]
            _dmac[0] += 1
            e.dma_start(out_ap, in_ap)

        # ---- constants ----
        ident_f = consts.tile([P, P], F32)
        make_identity(nc, ident_f)
        ident = consts.tile([P, P], FR)
        nc.gpsimd.dma_start(ident[:], ident_f[:])

        ones_f = consts.tile([1, P], F32)
        nc.gpsimd.memset(ones_f[:], 1.0)
        ones = consts.tile([1, P], FR)
        nc.gpsimd.dma_start(ones[:], ones_f[:])

        bv_sb = consts.tile([1, D], FR)
        nc.gpsimd.dma_start(bv_sb[:], bv[:].rearrange("(one d) -> one d", one=1))
        bq_sb = consts.tile([P, ET], F32)
        nc.sync.dma_start(bq_sb[:], bq[:].rearrange("(t p) -> p t", p=P))
        bk_sb = consts.tile([P, ET], F32)
        nc.sync.dma_start(bk_sb[:], bk[:].rearrange("(t p) -> p t", p=P))

        # bv broadcast to [P, D] via ones.T @ bv (K=1 matmul)
        bv_bcast = consts.tile([P, D], F32)
        for ec in range(EC):
            pstmp = ps512.tile([P, 512], F32, tag="ps512")
            nc.tensor.matmul(pstmp[:], ones[:], bv_sb[:, ec * 512:(ec + 1) * 512],
                             start=True, stop=True)
            nc.any.tensor_copy(bv_bcast[:, ec * 512:(ec + 1) * 512], pstmp[:])

        qT_dram = dram.tile([D, NQ], FR)
        kpool = ctx.enter_context(tc.tile_pool(name="kpool", bufs=1))
        kT = kpool.tile([P, ET, S], FR)      # [e-part, e-tile, j]

        # ---- Phase 1: Q projection (wk prefetched) ----
        wk_ctx = tc.tile_pool(name="wk", bufs=1)
        wkp = wk_ctx.__enter__()
        with tc.tile_pool(name="wq", bufs=1) as wqp, \
             tc.tile_pool(name="xq", bufs=1) as xqp, \
             tc.tile_pool(name="qo", bufs=2) as qop:
            wq_sb = wqp.tile([P, DT, D], FR)
            for dt in range(DT):
                dma(wq_sb[:, dt, :], wqT[dt * P:(dt + 1) * P, :])
            wk_sb = wkp.tile([P, DT, D], FR)
            for dt in range(DT):
                dma(wk_sb[:, dt, :], wkT[dt * P:(dt + 1) * P, :])
            for g in range(IG):
                xq_g = xqp.tile([P, DT, 512], FR, tag="xq")
                for dt in range(DT):
                    dma(xq_g[:, dt, :],
                                      xqT[dt * P:(dt + 1) * P, g * 512:(g + 1) * 512])
                for et in range(ET):
                    psq = ps512.tile([P, 512], F32, tag="ps512")
                    for dt in range(DT):
                        nc.tensor.matmul(psq[:], wq_sb[:, dt, et * P:(et + 1) * P],
                                         xq_g[:, dt, :], start=(dt == 0), stop=(dt == DT - 1))
                    qo = qop.tile([P, 512], FR, tag="qo")
                    nc.vector.tensor_scalar_add(qo[:], psq[:], bq_sb[:, et:et + 1])
                    dma(qT_dram[et * P:(et + 1) * P, g * 512:(g + 1) * 512], qo[:])

        # ---- Phase 2: K projection -> kT resident [e, j] ----
        with tc.tile_pool(name="xk", bufs=2) as xkp:
            for sc in range(SC):
                xk_g = xkp.tile([P, DT, 512], FR, tag="xk")
                for dt in range(DT):
                    dma(xk_g[:, dt, :],
                                      xkvT[dt * P:(dt + 1) * P, sc * 512:(sc + 1) * 512])
                for et in range(ET):
                    psk = ps512.tile([P, 512], F32, tag="ps512")
                    for dt in range(DT):
                        nc.tensor.matmul(psk[:], wk_sb[:, dt, et * P:(et + 1) * P],
                                         xk_g[:, dt, :], start=(dt == 0), stop=(dt == DT - 1))
                    nc.vector.tensor_scalar_add(kT[:, et, sc * 512:(sc + 1) * 512],
                                                psk[:], bk_sb[:, et:et + 1])

        # ---- Phase 3: V projection -> v resident [j, e] (no bias) ----
        wk_ctx.__exit__(None, None, None)
        vpool = ctx.enter_context(tc.tile_pool(name="vpool", bufs=1))
        vN = vpool.tile([P, SB, D], FR)      # [s-part, j-tile, e]
        with tc.tile_pool(name="wv", bufs=1) as wvp, \
             tc.tile_pool(name="xv", bufs=2) as xvp:
            wv_sb = wvp.tile([P, DT, D], FR)
            for dt in range(DT):
                dma(wv_sb[:, dt, :], wvT[dt * P:(dt + 1) * P, :])
            for sb_i in range(SB):
                xv_g = xvp.tile([P, DT, P], FR, tag="xv")
                for dt in range(DT):
                    dma(xv_g[:, dt, :],
                                      xkvT[dt * P:(dt + 1) * P, sb_i * P:(sb_i + 1) * P])
                for ec in range(EC):
                    psv = ps512.tile([P, 512], F32, tag="ps512")
                    for dt in range(DT):
                        nc.tensor.matmul(psv[:], xv_g[:, dt, :],
                                         wv_sb[:, dt, ec * 512:(ec + 1) * 512],
                                         start=(dt == 0), stop=(dt == DT - 1))
                    nc.any.tensor_copy(vN[:, sb_i, ec * 512:(ec + 1) * 512], psv[:])

        # ---- Phase 4: attention ----
        with tc.tile_pool(name="qg", bufs=1) as qgp, \
             tc.tile_pool(name="attn", bufs=3) as attnp, \
             tc.tile_pool(name="attnT", bufs=6) as attnTp, \
             tc.tile_pool(name="epi", bufs=2) as epip:
            for g in range(IG):
                qT_g = qgp.tile([P, ET, 512], FR, tag="qg")
                for et in range(ET):
                    dma(qT_g[:, et, :],
                                      qT_dram[et * P:(et + 1) * P, g * 512:(g + 1) * 512])
                for ib in range(4):
                    i0 = ib * P
                    out_ps = [outps.tile([P, 512], F32, tag=f"outps{ec}", name=f"out_ps{ec}")
                              for ec in range(EC)]
                    rs = epip.tile([P, SC], F32, tag="rs")
                    for jc in range(SC):
                        sc_ps = ps512.tile([P, 512], F32, tag="ps512")
                        for et in range(ET):
                            nc.tensor.matmul(sc_ps[:], qT_g[:, et, i0:i0 + P],
                                             kT[:, et, jc * 512:(jc + 1) * 512],
                                             start=(et == 0), stop=(et == ET - 1))
                        attn = attnp.tile([P, 512], FR, tag="attn")
                        nc.scalar.activation(attn[:], sc_ps[:], AF.Exp,
                                             scale=INV_SQRT_D, accum_out=rs[:, jc:jc + 1])
                        for jt in range(4):
                            jg = jc * 4 + jt
                            tps = tpps.tile([P, P], FR, tag="tps")
                            nc.tensor.transpose(tps[:], attn[:, jt * P:(jt + 1) * P], ident[:])
                            attnT = attnTp.tile([P, P], FR, tag="attnT")
                            nc.any.tensor_copy(attnT[:], tps[:])
                            for ec in range(EC):
                                nc.tensor.matmul(out_ps[ec][:], attnT[:],
                                                 vN[:, jg, ec * 512:(ec + 1) * 512],
                                                 start=(jg == 0), stop=(jg == SB - 1))
                    rsum = epip.tile([P, 1], F32, tag="rsum")
                    nc.vector.tensor_reduce(rsum[:], rs[:], mybir.AxisListType.X,
                                            mybir.AluOpType.add)
                    invs = epip.tile([P, 1], F32, tag="invs")
                    nc.vector.reciprocal(invs[:], rsum[:])
                    out_sb = epip.tile([P, D], F32, tag="out_sb")
                    for ec in range(EC):
                        nc.vector.tensor_scalar_mul(out_sb[:, ec * 512:(ec + 1) * 512],
                                                    out_ps[ec][:], invs[:])
                    nc.vector.tensor_add(out_sb[:], out_sb[:], bv_bcast[:])
                    r0 = g * 512 + i0
                    dma(out[r0:r0 + P, :], out_sb[:])

    nc.compile()
    return nc


def make_in_maps(x, Wq, bq, Wk, bk, Wv, bv):
    x = np.asarray(x, np.float32)
    wqT = np.ascontiguousarray(np.asarray(Wq, np.float32).T)
    wkT = np.ascontiguousarray(np.asarray(Wk, np.float32).T)
    wvT = np.ascontiguousarray(np.asarray(Wv, np.float32).T)
    bq = np.ascontiguousarray(np.asarray(bq, np.float32))
    bk = np.ascontiguousarray(np.asarray(bk, np.float32))
    bv = np.ascontiguousarray(np.asarray(bv, np.float32))
    in_maps = []
    for c in range(NCORES):
        b, h = c // 2, c % 2
        xb = x[b]
        in_maps.append({
            "xkvT": np.ascontiguousarray(xb.T),
            "xqT": np.ascontiguousarray(xb[h * NQ:(h + 1) * NQ].T),
            "wqT": wqT, "wkT": wkT, "wvT": wvT,
            "bq": bq, "bk": bk, "bv": bv,
        })
    return in_maps


def get_nc():
    if "nc" not in _CACHE:
        _CACHE["nc"] = build_nc()
    return _CACHE["nc"]


def kernel(x, Wq, bq, Wk, bk, Wv, bv):
    from concourse.bass_utils import run_bass_kernel_spmd
    nc = get_nc()
    in_maps = make_in_maps(x, Wq, bq, Wk, bk, Wv, bv)
    res = run_bass_kernel_spmd(nc, in_maps, core_ids=list(range(NCORES)))
    out = np.empty((B, S, D), np.float32)
    for c in range(NCORES):
        b, h = c // 2, c % 2
        out[b, h * NQ:(h + 1) * NQ] = res.results[c]["out"]
    return out



# revision 4
# speedup vs baseline: 1.7861x; 1.7861x over previous
"""Single-head attention (InterModalAttention) Bass kernel for 8 TRN2 cores.

Sharding: batch (4) x query-half (2) -> 8 cores; each core owns 1024 queries
of one batch element and attends over the full 2048-key sequence.

Algebraic reformulation (removes K/V projections and all PE transposes):
  scores[i,j] = q_i . k_j with q = x Wq^T + bq, k = x Wk^T + bk.
  The j-constant terms (q_i . bk) cancel in softmax, so with
      M  = Wq^T Wk        (host-precomputed, [d, d'])
      bu = bq  Wk         (host-precomputed, [d'])
      u  = x_q M + bu     (on-device "U projection", [i, d'])
  scoresT[j, i] = sum_d' x[j, d'] u[i, d']  -- lhsT = xT (raw input!).
  Output side:  out = attn v + bv = (attn x) Wv^T + bv:
      attnXT[d, i] = sum_j xN[j, d] attnT[j, i]   (attnT = exp(scoresT/32))
      out[i, e]    = sum_d attnXT[d, i] WvT[d, e] * inv_rowsum_i + bv[e]
  Row sums via ones-lhsT matmul on attnT; normalization folded into epilogue.

All matmul operands fp16 (PSUM accumulation fp32; verified rel err ~3.5e-4).
Per-core PE load ~410k rows vs ~616k for the direct q/k/v formulation.
"""
import sys
import numpy as np

for p in ("/opt/trn_rl_repo",):
    if p not in sys.path:
        sys.path.insert(0, p)

B, S, D = 4, 2048, 1024
NQ = 1024          # queries per core
NCORES = 8
P = 128
INV_SQRT_D = 1.0 / 32.0

_CACHE = {}


def build_nc():
    from contextlib import ExitStack
    import concourse.mybir as mybir
    import concourse.tile as tile
    from concourse import bacc

    F32 = mybir.dt.float32
    F16 = mybir.dt.float16
    AF = mybir.ActivationFunctionType

    nc = bacc.Bacc("TRN2", debug=False)

    xqT = nc.dram_tensor("xqT", (D, NQ), F16, kind="ExternalInput")
    xT = nc.dram_tensor("xT", (D, S), F16, kind="ExternalInput")
    xN = nc.dram_tensor("xN", (S, D), F16, kind="ExternalInput")
    m = nc.dram_tensor("m", (D, D), F16, kind="ExternalInput")
    wvT = nc.dram_tensor("wvT", (D, D), F16, kind="ExternalInput")
    bu = nc.dram_tensor("bu", (D,), F32, kind="ExternalInput")
    bv16 = nc.dram_tensor("bv16", (D,), F16, kind="ExternalInput")
    out = nc.dram_tensor("out", (NQ, D), F32, kind="ExternalOutput")

    DT = D // P            # 8 d-tiles
    SB = S // P            # 16 j-tiles
    IG = NQ // 512         # 2 i-chunks
    EC = D // 512          # 2 e-chunks
    IB = 512 // P          # 4 i-subtiles per chunk

    with tile.TileContext(nc) as tc, ExitStack() as ctx:
        consts = ctx.enter_context(tc.tile_pool(name="consts", bufs=1))
        xt_pool = ctx.enter_context(tc.tile_pool(name="xt", bufs=1))
        xn_pool = ctx.enter_context(tc.tile_pool(name="xn", bufs=1))
        wv_pool = ctx.enter_context(tc.tile_pool(name="wv", bufs=1))
        u_pool = ctx.enter_context(tc.tile_pool(name="u", bufs=1))
        dram = ctx.enter_context(tc.tile_pool(name="dram", bufs=2, space="DRAM"))

        _ld = [nc.sync, nc.gpsimd, nc.scalar]
        _ldc = [0]
        def dma(out_ap, in_ap):
            e = _ld[_ldc[0] % len(_ld)]
            _ldc[0] += 1
            e.dma_start(out_ap, in_ap)

        _st = [nc.sync, nc.gpsimd]
        _stc = [0]
        def dma2(out_ap, in_ap):
            e = _st[_stc[0] % len(_st)]
            _stc[0] += 1
            e.dma_start(out_ap, in_ap)

        # ---- constants ----
        ones_col = consts.tile([P, 1], F16)
        nc.gpsimd.memset(ones_col[:], 1.0)
        ones_row = consts.tile([1, P], F16)
        nc.gpsimd.memset(ones_row[:], 1.0)
        bu_sb = consts.tile([P, DT], F32)
        nc.sync.dma_start(bu_sb[:], bu[:].rearrange("(t p) -> p t", p=P))
        bv_sb = consts.tile([1, D], F16)
        nc.sync.dma_start(bv_sb[:], bv16[:].rearrange("(one d) -> one d", one=1))
        bv_bcast = consts.tile([P, D], F32)

        xT_sb = xt_pool.tile([P, DT, S], F16)
        xN_sb = xn_pool.tile([P, SB, D], F16)
        wv_sb = wv_pool.tile([P, DT, D], F16)
        uT_sb = u_pool.tile([P, DT, NQ], F16)

        # ---- Phase U: u = xq M + bu, streamed against the input DMAs ----
        with tc.tile_pool(name="mp", bufs=1) as mp, \
             tc.tile_pool(name="xqp", bufs=1) as xqp, \
             tc.tile_pool(name="ups", bufs=1, space="PSUM") as ups:
            m_sb = mp.tile([P, DT, D], F16)
            xq_sb = xqp.tile([P, DT, NQ], F16)
            # load order: (m[dt], xq_g0[dt]) pairs first so the dt-outer
            # U accumulation starts ~1.5us in and streams with the DMAs
            for dt in range(DT):
                dma(m_sb[:, dt, :], m[dt * P:(dt + 1) * P, :])
                dma(xq_sb[:, dt, 0:512], xqT[dt * P:(dt + 1) * P, 0:512])
            for dt in range(DT):
                dma(xq_sb[:, dt, 512:1024], xqT[dt * P:(dt + 1) * P, 512:1024])
            for dt in range(DT):
                dma(xT_sb[:, dt, :], xT[dt * P:(dt + 1) * P, :])
            for jb in range(SB):
                dma(xN_sb[:, jb, :], xN[jb * P:(jb + 1) * P, :])
            for dt in range(DT):
                dma(wv_sb[:, dt, :], wvT[dt * P:(dt + 1) * P, :])

            for g in range(IG):
                ups_t = [ups.tile([P, 512], F32, tag=f"ups{dc}", name=f"ups{dc}")
                         for dc in range(DT)]
                for dt in range(DT):
                    for dc in range(DT):
                        nc.tensor.matmul(ups_t[dc][:], m_sb[:, dt, dc * P:(dc + 1) * P],
                                         xq_sb[:, dt, g * 512:(g + 1) * 512],
                                         start=(dt == 0), stop=(dt == DT - 1))
                for dc in range(DT):
                    nc.vector.tensor_scalar_add(uT_sb[:, dc, g * 512:(g + 1) * 512],
                                                ups_t[dc][:], bu_sb[:, dc:dc + 1])

        # ---- attention-phase pools (8 PSUM banks total) ----
        ps = ctx.enter_context(tc.tile_pool(name="ps", bufs=3, space="PSUM"))
        axps = ctx.enter_context(tc.tile_pool(name="axps", bufs=2, space="PSUM"))
        outps = ctx.enter_context(tc.tile_pool(name="outps", bufs=2, space="PSUM"))
        rsps = ctx.enter_context(tc.tile_pool(name="rsps", bufs=1, space="PSUM"))
        attnp = ctx.enter_context(tc.tile_pool(name="attn", bufs=2))
        axp = ctx.enter_context(tc.tile_pool(name="ax", bufs=2))
        epip = ctx.enter_context(tc.tile_pool(name="epi", bufs=2))

        # bv broadcast [P, D] via ones_row.T @ bv (K=1 matmul)
        for ec in range(EC):
            pstmp = ps.tile([P, 512], F32, tag="ps")
            nc.tensor.matmul(pstmp[:], ones_row[:], bv_sb[:, ec * 512:(ec + 1) * 512],
                             start=True, stop=True)
            nc.vector.tensor_copy(bv_bcast[:, ec * 512:(ec + 1) * 512], pstmp[:])

        for g in range(IG):
            i0, i1 = g * 512, (g + 1) * 512
            attnT = attnp.tile([P, SB, 512], F16, tag="attnT")
            rsp = rsps.tile([1, 512], F32, tag="rs")
            # alpha: scoresT tiles + exp; rowsum matmul lags one jb so the
            # PE never waits on the ACT engine
            for jb in range(SB):
                scp = ps.tile([P, 512], F32, tag="ps")
                for dc in range(DT):
                    nc.tensor.matmul(scp[:], xT_sb[:, dc, jb * P:(jb + 1) * P],
                                     uT_sb[:, dc, i0:i1],
                                     start=(dc == 0), stop=(dc == DT - 1))
                nc.scalar.activation(attnT[:, jb, :], scp[:], AF.Exp, scale=INV_SQRT_D)
                if jb > 0:
                    nc.tensor.matmul(rsp[:], ones_col[:], attnT[:, jb - 1, :],
                                     start=(jb == 1), stop=False)
            # beta: attnXT[d, i] accumulation over j
            axT = axp.tile([P, DT, 512], F16, tag="axT")
            for dt in range(DT):
                axpt = axps.tile([P, 512], F32, tag="axps")
                for jb in range(SB):
                    nc.tensor.matmul(axpt[:], xN_sb[:, jb, dt * P:(dt + 1) * P],
                                     attnT[:, jb, :],
                                     start=(jb == 0), stop=(jb == SB - 1))
                if dt == 0:
                    nc.tensor.matmul(rsp[:], ones_col[:], attnT[:, SB - 1, :],
                                     start=False, stop=True)
                nc.vector.tensor_copy(axT[:, dt, :], axpt[:])
            # rowsum -> per-partition inverse (via DRAM bounce transpose)
            rs_row = epip.tile([1, 512], F32, tag="rs_row")
            nc.vector.tensor_copy(rs_row[:], rsp[:])
            rs_dram = dram.tile([512], F32, tag="rs_dram")
            dma2(rs_dram[:], rs_row[:])
            rs_col = epip.tile([P, IB], F32, tag="rs_col")
            dma2(rs_col[:], rs_dram[:].rearrange("(i p) -> p i", p=P))
            inv = epip.tile([P, IB], F32, tag="inv")
            nc.vector.reciprocal(inv[:], rs_col[:])
            # gamma: out[i, e] = attnXT.T @ WvT, normalized + bv
            for ib in range(IB):
                out_sb = epip.tile([P, D], F32, tag="out_sb")
                for ec in range(EC):
                    op = outps.tile([P, 512], F32, tag="outps")
                    for dt in range(DT):
                        nc.tensor.matmul(op[:], axT[:, dt, ib * P:(ib + 1) * P],
                                         wv_sb[:, dt, ec * 512:(ec + 1) * 512],
                                         start=(dt == 0), stop=(dt == DT - 1))
                    nc.vector.tensor_scalar_mul(out_sb[:, ec * 512:(ec + 1) * 512],
                                                op[:], inv[:, ib:ib + 1])
                nc.vector.tensor_add(out_sb[:], out_sb[:], bv_bcast[:])
                r0 = g * 512 + ib * P
                dma2(out[r0:r0 + P, :], out_sb[:])

    nc.compile()
    return nc


def make_in_maps(x, Wq, bq, Wk, bk, Wv, bv):
    x = np.asarray(x, np.float32)
    Wq = np.asarray(Wq, np.float32)
    Wk = np.asarray(Wk, np.float32)
    Wv = np.asarray(Wv, np.float32)
    m16 = np.ascontiguousarray((Wq.T @ Wk).astype(np.float16))
    bu32 = np.ascontiguousarray((np.asarray(bq, np.float32) @ Wk).astype(np.float32))
    wvT16 = np.ascontiguousarray(Wv.T.astype(np.float16))
    bv16 = np.ascontiguousarray(np.asarray(bv, np.float32).astype(np.float16))
    in_maps = []
    for c in range(NCORES):
        b, h = c // 2, c % 2
        xb = x[b]
        xbT16 = np.ascontiguousarray(xb.T.astype(np.float16))
        in_maps.append({
            "xqT": np.ascontiguousarray(xbT16[:, h * NQ:(h + 1) * NQ]),
            "xT": xbT16,
            "xN": np.ascontiguousarray(xb.astype(np.float16)),
            "m": m16, "wvT": wvT16,
            "bu": bu32, "bv16": bv16,
        })
    return in_maps


def get_nc():
    if "nc" not in _CACHE:
        _CACHE["nc"] = build_nc()
    return _CACHE["nc"]


def kernel(x, Wq, bq, Wk, bk, Wv, bv):
    from concourse.bass_utils import run_bass_kernel_spmd
    nc = get_nc()
    in_maps = make_in_maps(x, Wq, bq, Wk, bk, Wv, bv)
    res = run_bass_kernel_spmd(nc, in_maps, core_ids=list(range(NCORES)))
    out = np.empty((B, S, D), np.float32)
    for c in range(NCORES):
        b, h = c // 2, c % 2
        out[b, h * NQ:(h + 1) * NQ] = res.results[c]["out"]
    return out
